# revision 1
# baseline (speedup 1.0000x reference)
"""AllAtomE3Encoder on 8 TRN2 NeuronCores (Bass/Tile, graph-parallel).

Sharding: atoms/residues in contiguous blocks of 5632 atoms / 256 residues per
core (residue-aligned); edges partitioned by destination atom, sorted by
(src-half, dst-window-of-128-atoms) and padded per block to a common per-core
tile count so all cores run one SPMD graph.

Per layer: hs = h@We1[:H] is computed locally, AllGathered into a DRAM table,
and the per-edge hs[src] fetched with SWDGE dma_gather (int16 indices; the
src>=32768 half uses a table-offset second region).  hd[dst] broadcast and the
dst segment-sum both go through one-hot window matmuls on TensorE.  The edge
MLP runs in a transposed (feature-on-partition) layout in bf16; node MLP and
the segment-softmax pooling run in f32.
"""
import os
import sys
import numpy as np

for _p in ("/opt/trn_rl_repo",):
    if _p not in sys.path and os.path.isdir(_p):
        sys.path.insert(0, _p)

from ml_dtypes import bfloat16

from concourse import bacc, bass, mybir, tile
from concourse.bass_utils import run_bass_kernel_spmd
from concourse.library_config import mlp as _mlp_lib
from concourse._compat import get_trn_type

DT = mybir.dt
AF = mybir.ActivationFunctionType
ALU = mybir.AluOpType

P = 128
C = 8
H = 128
R = 16
L = 3
LAT = 32
N_RES = 2048
CUTOFF = 5.0
NLOC = N_RES // C            # 256 residues / core
A = 45056
ALOC = A // C                # 5632 atoms / core
NT = ALOC // P               # 44 atom tiles / core (= dst windows)
NTH = NT // 2                # tiles per residue window
HALF = 32768                 # int16 index split
GC = int(os.environ.get("KGC", "1024"))                    # gather chunk (edges per dma_gather call)
SQ = float(H) ** -0.5

_cache = {}


# --------------------------------------------------------------------------
# host-side preprocessing
# --------------------------------------------------------------------------

def _prep(inputs):
    f32 = np.float32
    coords = np.asarray(inputs["atom_coords"], f32)
    coords = coords - coords.mean(0, keepdims=True)
    src = np.asarray(inputs["edge_src"]).astype(np.int64)
    dst = np.asarray(inputs["edge_dst"]).astype(np.int64)
    atype = np.asarray(inputs["atom_types"]).astype(np.int64)
    ridx = np.asarray(inputs["residue_indices"]).astype(np.int64)
    rtype = np.asarray(inputs["residue_types"]).astype(np.int64)

    d = np.linalg.norm(coords[src] - coords[dst], axis=-1).astype(f32)
    centers = np.linspace(0.0, CUTOFF, R).astype(f32)
    gamma = (R / CUTOFF) ** 2
    ea = np.exp(-gamma * (d[:, None] - centers) ** 2).astype(f32)   # (E,16)

    core = dst // ALOC
    win = (dst - core * ALOC) // P
    half = (src >= HALF).astype(np.int64)

    counts = np.zeros((C, 2, NT), np.int64)
    eids = [[[None] * NT for _ in range(2)] for _ in range(C)]
    for c in range(C):
        m_c = np.nonzero(core == c)[0]
        hw = half[m_c] * NT + win[m_c]
        o = np.argsort(hw, kind="stable")
        m_c = m_c[o]
        hw = hw[o]
        b = np.searchsorted(hw, np.arange(2 * NT + 1))
        for hh in range(2):
            for w in range(NT):
                k = hh * NT + w
                eids[c][hh][w] = m_c[b[k]:b[k + 1]]
                counts[c, hh, w] = b[k + 1] - b[k]

    tiles = np.maximum(1, -(-counts.max(axis=0) // P))      # (2, NT) shared
    assert tiles.max() * P <= 2560, tiles.max()
    blk_off = np.zeros((2, NT), np.int64)
    pos = 0
    for hh in range(2):
        for w in range(NT):
            blk_off[hh, w] = pos
            pos += tiles[hh, w] * P
    e_pad = int(pos)
    e_lo = int(blk_off[1, 0])

    chunks = []
    for (start, end) in ((0, e_lo), (e_lo, e_pad)):
        p0 = start
        while p0 < end:
            n = min(GC, end - p0)
            chunks.append((p0, n, 1 if start == e_lo else 0))
            p0 += n

    per_core = []
    bf = bfloat16
    for c in range(C):
        sdst = np.zeros((P, e_pad), bf)
        sdstT = np.zeros((P, e_pad // P, P), bf)
        eaT = np.zeros((16, e_pad), bf)
        srcrel = np.zeros(e_pad, np.int16)
        for hh in range(2):
            for w in range(NT):
                ids = eids[c][hh][w]
                n = len(ids)
                if n == 0:
                    continue
                o = int(blk_off[hh, w])
                a_rel = (dst[ids] - c * ALOC - w * P).astype(np.int64)
                col = o + np.arange(n)
                sdst[a_rel, col] = 1
                sdstT[col % P, col // P, a_rel] = 1
                eaT[:, col] = ea[ids].T.astype(bf)
                srcrel[col] = (src[ids] - HALF * hh).astype(np.int16)
        gidx = np.tile(srcrel.reshape(e_pad // 16, 16).T, (8, 1)).astype(np.int16)

        sl_a = slice(c * ALOC, (c + 1) * ALOC)
        sl_r = slice(c * NLOC, (c + 1) * NLOC)
        at_c = atype[sl_a]
        rt_atom_c = rtype[ridx[sl_a]]
        rloc = ridx[sl_a] - c * NLOC
        oh_atomT = np.zeros((64, ALOC), f32); oh_atomT[at_c, np.arange(ALOC)] = 1
        ohres4T = np.zeros((4, ALOC), f32); ohres4T[rt_atom_c, np.arange(ALOC)] = 1
        oh4T_res = np.zeros((4, NLOC), f32); oh4T_res[rtype[sl_r], np.arange(NLOC)] = 1

        rrel = rloc % P
        aloc_i = np.arange(ALOC)
        t_i = aloc_i // P
        a_i = aloc_i % P
        sres_g = np.zeros((P, NT, P), bf); sres_g[rrel, t_i, a_i] = 1
        sres_s = np.zeros((P, NT, P), bf); sres_s[a_i, t_i, rrel] = 1
        apr = np.asarray(inputs["atoms_per_residue"]).astype(np.int64)[sl_r]
        starts = np.concatenate([[0], np.cumsum(apr)[:-1]])
        slot = aloc_i - starts[rloc]
        assert slot.max() < 32
        slot32 = np.zeros((P, NT, 32), bf); slot32[a_i, t_i, slot] = 1
        padmask = np.where(np.arange(32)[None, :] < apr[:, None], 0.0, -1e30).astype(f32)
        padmask2 = np.concatenate([padmask[:P], padmask[P:]], axis=1)  # [128, 64]

        per_core.append(dict(
            sdst=np.ascontiguousarray(sdst),
            sdstT=np.ascontiguousarray(sdstT),
            eaT=np.ascontiguousarray(eaT),
            gidx=np.ascontiguousarray(gidx),
            oh_atomT=oh_atomT, ohres4T=ohres4T, oh4T_res=oh4T_res,
            sres_g=np.ascontiguousarray(sres_g.reshape(P, NT * P)),
            sres_s=np.ascontiguousarray(sres_s.reshape(P, NT * P)),
            slot32=np.ascontiguousarray(slot32.reshape(P, NT * 32)),
            padmask2=np.ascontiguousarray(padmask2),
        ))

    We1 = np.asarray(inputs["We1"], f32)
    Wh1 = np.asarray(inputs["Wh1"], f32)
    wshared = dict(
        atom_embed=np.asarray(inputs["atom_embed"], f32),
        residue_embed=np.asarray(inputs["residue_embed"], f32),
        ws=np.ascontiguousarray(We1[:, :H, :].transpose(1, 0, 2)),
        wd=np.ascontiguousarray(We1[:, H:2 * H, :].transpose(1, 0, 2)),
        wrbf_bf=np.ascontiguousarray(We1[:, 2 * H:, :].transpose(1, 0, 2).astype(bf)),
        we2_bf=np.ascontiguousarray(np.asarray(inputs["We2"], f32).transpose(1, 0, 2)).astype(bf),
        be1T=np.ascontiguousarray(np.asarray(inputs["be1"], f32).T),
        be2T=np.ascontiguousarray(np.asarray(inputs["be2"], f32).T),
        wh1h=np.ascontiguousarray(Wh1[:, :H, :].transpose(1, 0, 2)),
        wh1a=np.ascontiguousarray(Wh1[:, H:, :].transpose(1, 0, 2)),
        wh2=np.ascontiguousarray(np.asarray(inputs["Wh2"], f32).transpose(1, 0, 2)),
        bh1T=np.ascontiguousarray(np.asarray(inputs["bh1"], f32).T),
        bh2T=np.ascontiguousarray(np.asarray(inputs["bh2"], f32).T),
        wq=np.asarray(inputs["Wq"], f32), wk=np.asarray(inputs["Wk"], f32),
        wv=np.asarray(inputs["Wv"], f32),
        bq_row=np.asarray(inputs["bq"], f32)[None, :],
        bk_row=np.asarray(inputs["bk"], f32)[None, :],
        bv_row=np.asarray(inputs["bv"], f32)[None, :],
        wmu=np.asarray(inputs["Wmu"], f32), wlv=np.asarray(inputs["Wlv"], f32),
        bmu_col=np.asarray(inputs["bmu"], f32)[:, None],
        blv_col=np.asarray(inputs["blv"], f32)[:, None],
        ones1=np.ones((1, P), f32),
        ident_bf=np.eye(P, dtype=bf),
        ident_f=np.eye(P, dtype=f32),
    )

    meta = dict(tiles=tiles, blk_off=blk_off, e_pad=e_pad, chunks=chunks)
    return meta, per_core, wshared


# --------------------------------------------------------------------------
# device graph
# --------------------------------------------------------------------------

_NPDT = {np.dtype(np.float32): DT.float32,
         np.dtype(bfloat16): DT.bfloat16,
         np.dtype(np.int16): DT.int16}

_PERSIST = ("atom_embed", "residue_embed", "ws", "wd", "wrbf_bf", "we2_bf",
            "be1T", "be2T", "wh1h", "wh1a", "wh2", "bh1T", "bh2T",
            "wq", "wk", "wv", "bq_row", "bk_row", "bv_row",
            "wmu", "wlv", "bmu_col", "blv_col", "ones1", "ident_bf",
            "ident_f", "gidx", "oh4T_res", "sres_g", "sres_s", "slot32", "padmask2")


def _build(meta, shapes):
    nc = bacc.Bacc(get_trn_type() or "TRN2", target_bir_lowering=False)
    tiles = meta["tiles"]
    blk_off = meta["blk_off"]
    chunks = meta["chunks"]

    ins = {}
    for name, arr in shapes.items():
        ins[name] = nc.declare_dram_parameter(
            name, list(arr.shape), _NPDT[arr.dtype], isOutput=False)
    out_ext = nc.declare_dram_parameter("out", [64, NLOC], DT.float32, isOutput=True)

    hs_loc = [nc.dram_tensor(f"hs_loc{l}", [NT, P, H], DT.bfloat16) for l in range(L)]
    hs_full = [nc.dram_tensor(f"hs_full{l}", [A, H], DT.bfloat16, addr_space="Shared")
               for l in range(L)]

    with tile.TileContext(nc) as tc:
        with tc.tile_pool(name="persist", bufs=1) as pp, \
             tc.tile_pool(name="work", bufs=2) as wp, \
             tc.tile_pool(name="psum", bufs=1, space="PSUM") as ps:
            nc.gpsimd.load_library(_mlp_lib)

            w_sb = {}
            for name in _PERSIST:
                arr = shapes[name]
                t = pp.tile(list(arr.shape), _NPDT[arr.dtype], name=f"sb_{name}")
                nc.sync.dma_start(t[:], ins[name][:])
                w_sb[name] = t

            out_st = pp.tile([64, NLOC], DT.float32, name="out_st")
            nc.vector.memset(out_st[:], 0.0)
            kstop = set(os.environ.get("KSTOP", "").split(","))

            hT = pp.tile([P, NT * P], DT.float32, name="hT")
            aggT = pp.tile([P, NT * P], DT.float32, name="aggT")
            hd_hi = pp.tile([P, NT * P], DT.bfloat16, name="hd_hi")

            # ---- h0 (H-part): atom_embed one-hot + residue_embed one-hot
            for t0 in range(0, NT, 4):
                n = min(4, NT - t0) * P
                sl = slice(t0 * P, t0 * P + n)
                oha = wp.tile([64, 512], DT.float32, tag="oha", bufs=2)
                nc.sync.dma_start(oha[:, :n], ins["oh_atomT"][:, sl])
                ohr = wp.tile([4, 512], DT.float32, tag="ohr", bufs=2)
                nc.sync.dma_start(ohr[:, :n], ins["ohres4T"][:, sl])
                pa = ps.tile([P, 512], DT.float32, tag="pw1", bufs=2)
                nc.tensor.matmul(pa[:, :n], lhsT=w_sb["atom_embed"][:],
                                 rhs=oha[:, :n], start=True, stop=False)
                nc.tensor.matmul(pa[:, :n], lhsT=w_sb["residue_embed"][:],
                                 rhs=ohr[:, :n], start=False, stop=True)
                nc.scalar.activation(hT[:, sl], pa[:, :n], AF.Copy)

            cut = bool(kstop & {"h0", "ag", "gather", "win"})

            # ---- layers
            nlayers = 0 if "h0" in kstop else int(os.environ.get("KLAYERS", str(L)))
            for l in range(nlayers):
                for t in range(NT):
                    tsl = slice(t * P, (t + 1) * P)
                    ph = ps.tile([P, P], DT.float32, tag="psq1", bufs=2)
                    nc.tensor.matmul(ph[:], lhsT=hT[:, tsl], rhs=w_sb["ws"][:, l, :],
                                     start=True, stop=True)
                    hs_t = wp.tile([P, P], DT.bfloat16, tag="hs_t", bufs=3)
                    nc.vector.tensor_copy(hs_t[:], ph[:])
                    nc.sync.dma_start(hs_loc[l][t], hs_t[:])
                    pd = ps.tile([P, P], DT.float32, tag="psq2", bufs=2)
                    nc.tensor.matmul(pd[:], lhsT=hT[:, tsl], rhs=w_sb["wd"][:, l, :],
                                     start=True, stop=True)
                    nc.vector.tensor_copy(hd_hi[:, tsl], pd[:])

                if "noag" not in kstop:
                    nc.gpsimd.collective_compute(
                        "AllGather", ALU.bypass,
                        replica_groups=[list(range(C))],
                        ins=[hs_loc[l][:].opt()], outs=[hs_full[l][:].opt()])
                else:
                    nc.sync.dma_start(hs_full[l][0:ALOC, :].opt(), hs_loc[l][:].opt())
                if "ag" in kstop:
                    continue

                ghs = {}
                _kch = int(os.environ.get("KCHUNKS", "0"))
                for (p0, n, hh) in (chunks[:_kch] if _kch else chunks):
                    g = wp.tile([P, GC // P, P], DT.bfloat16, tag="ghs", bufs=4)
                    src_ap = hs_full[l][HALF:, :] if hh else hs_full[l][:]
                    nc.gpsimd.dma_gather(
                        out_ap=g[:, 0:n // P, :], in_ap=src_ap,
                        idxs_ap=w_sb["gidx"][:, p0 // 16:(p0 + n) // 16],
                        num_idxs=n, num_idxs_reg=n, elem_size=H)
                    ghs[p0] = g

                if "gather" in kstop:
                    continue

                def chunk_of(pos):
                    for (p0, n, hh) in chunks:
                        if p0 <= pos < p0 + n:
                            return p0, n
                    raise AssertionError(pos)

                for hh in range(2):
                    for w in range(NT):
                        nb = int(tiles[hh, w]) * P
                        b0 = int(blk_off[hh, w])
                        wsl = slice(w * P, (w + 1) * P)
                        sd = wp.tile([P, 2560], DT.bfloat16, tag="sdst", bufs=3)
                        nc.sync.dma_start(sd[:, :nb], ins["sdst"][:, b0:b0 + nb])
                        sdT = wp.tile([P, 20, P], DT.bfloat16, tag="sdstT", bufs=3)
                        nc.gpsimd.dma_start(sdT[:, :nb // P, :],
                                            ins["sdstT"][:, b0 // P:(b0 + nb) // P, :])
                        eat = wp.tile([16, 2560], DT.bfloat16, tag="eaT", bufs=2)
                        nc.sync.dma_start(eat[:, :nb], ins["eaT"][:, b0:b0 + nb])

                        m1T = wp.tile([P, 2560], DT.bfloat16, tag="m1T", bufs=2)
                        pos = b0
                        while pos < b0 + nb:
                            g0, gn = chunk_of(pos)
                            cn = min(512, b0 + nb - pos, g0 + gn - pos)
                            off = pos - b0
                            pm1 = ps.tile([P, 512], DT.float32, tag="pw1", bufs=2)
                            nc.tensor.matmul(pm1[:, :cn], lhsT=hd_hi[:, wsl],
                                             rhs=sd[:, off:off + cn],
                                             start=True, stop=False)
                            nc.tensor.matmul(pm1[:, :cn], lhsT=w_sb["wrbf_bf"][:, l, :],
                                             rhs=eat[:, off:off + cn],
                                             start=False, stop=True)
                            g = ghs[g0]
                            for j in range(cn // P):
                                jj = (pos - g0) // P + j
                                nc.tensor.matmul(
                                    pm1[:, j * P:(j + 1) * P],
                                    lhsT=g[:, jj, :], rhs=w_sb["ident_bf"][:],
                                    start=False, stop=False,
                                    skip_group_check=True)
                            nc.scalar.activation(m1T[:, off:off + cn], pm1[:, :cn],
                                                 AF.Silu, bias=w_sb["be1T"][:, l:l + 1])
                            pos += cn

                        m2T = wp.tile([P, 2560], DT.bfloat16, tag="m2T", bufs=2)
                        for off in range(0, nb, 512):
                            cn = min(512, nb - off)
                            pm2 = ps.tile([P, 512], DT.float32, tag="pw2", bufs=2)
                            nc.tensor.matmul(pm2[:, :cn], lhsT=w_sb["we2_bf"][:, l, :],
                                             rhs=m1T[:, off:off + cn],
                                             start=True, stop=True)
                            nc.scalar.activation(m2T[:, off:off + cn], pm2[:, :cn],
                                                 AF.Silu, bias=w_sb["be2T"][:, l:l + 1])

                        pagg = ps.tile([P, P], DT.float32, tag="psq1", bufs=2)
                        njt = nb // P
                        for j4 in range(0, njt, 4):
                            jn = min(4, njt - j4)
                            ptr = ps.tile([P, 4 * P], DT.bfloat16, tag="psq2", bufs=2)
                            for j in range(j4, j4 + jn):
                                nc.tensor.transpose(ptr[:, (j - j4) * P:(j - j4 + 1) * P],
                                                    in_=m2T[:, j * P:(j + 1) * P],
                                                    identity=w_sb["ident_bf"][:])
                            m2e = wp.tile([P, 4 * P], DT.bfloat16, tag="m2e", bufs=3)
                            nc.vector.tensor_copy(m2e[:, :jn * P], ptr[:, :jn * P])
                            for j in range(j4, j4 + jn):
                                nc.tensor.matmul(pagg[:],
                                                 lhsT=m2e[:, (j - j4) * P:(j - j4 + 1) * P],
                                                 rhs=sdT[:, j, :],
                                                 start=(j == 0), stop=(j == njt - 1))
                        if hh == 0:
                            nc.vector.tensor_copy(aggT[:, wsl], pagg[:])
                        else:
                            nc.vector.tensor_tensor(aggT[:, wsl], in0=aggT[:, wsl],
                                                    in1=pagg[:], op=ALU.add)

                # node MLP (f32)
                for t0 in range(0, NT, 4):
                    n = min(4, NT - t0) * P
                    sl = slice(t0 * P, t0 * P + n)
                    pu = ps.tile([P, 512], DT.float32, tag="pw1", bufs=2)
                    nc.tensor.matmul(pu[:, :n], lhsT=w_sb["wh1h"][:, l, :],
                                     rhs=hT[:, sl], start=True, stop=False)
                    nc.tensor.matmul(pu[:, :n], lhsT=w_sb["wh1a"][:, l, :],
                                     rhs=aggT[:, sl], start=False, stop=True)
                    uT = wp.tile([P, 512], DT.float32, tag="uT", bufs=2)
                    nc.scalar.activation(uT[:, :n], pu[:, :n], AF.Silu,
                                         bias=w_sb["bh1T"][:, l:l + 1])
                    ph2 = ps.tile([P, 512], DT.float32, tag="pw2", bufs=2)
                    nc.tensor.matmul(ph2[:, :n], lhsT=w_sb["wh2"][:, l, :],
                                     rhs=uT[:, :n], start=True, stop=False)
                    nc.tensor.matmul(ph2[:, :n], lhsT=w_sb["ident_f"][:],
                                     rhs=hT[:, sl], start=False, stop=True)
                    nc.scalar.activation(hT[:, sl], ph2[:, :n], AF.Identity,
                                         bias=w_sb["bh2T"][:, l:l + 1])

            # ---- pooling ----------------------------------------------------
            if not cut:
                pre = ps.tile([P, NLOC], DT.float32, tag="pw1", bufs=2)
                nc.tensor.matmul(pre[:], lhsT=w_sb["residue_embed"][:],
                                 rhs=w_sb["oh4T_res"][:], start=True, stop=True)
                res_embT = pp.tile([P, NLOC], DT.float32, name="res_embT")
                nc.scalar.activation(res_embT[:], pre[:], AF.Copy)
                q_sb = pp.tile([P, 2 * P], DT.float32, name="q_sb")
                for wi in range(2):
                    pq = ps.tile([P, P], DT.float32, tag="psq1", bufs=2)
                    nc.tensor.matmul(pq[:], lhsT=res_embT[:, wi * P:(wi + 1) * P],
                                     rhs=w_sb["wq"][:], start=True, stop=False)
                    nc.tensor.matmul(pq[:], lhsT=w_sb["ones1"][:], rhs=w_sb["bq_row"][:],
                                     start=False, stop=True)
                    nc.scalar.activation(q_sb[:, wi * P:(wi + 1) * P], pq[:], AF.Copy)
                q_hi = pp.tile([P, 2 * P], DT.bfloat16, name="q_hi")
                q_lo = pp.tile([P, 2 * P], DT.bfloat16, name="q_lo")
                nc.scalar.activation(q_hi[:], q_sb[:], AF.Copy)
                nc.vector.scalar_tensor_tensor(
                    q_lo[:], in0=q_sb[:], scalar=1.0, in1=q_hi[:],
                    op0=ALU.mult, op1=ALU.subtract)

                raw_st = pp.tile([P, NT], DT.float32, name="raw_st")
                negsm = pp.tile([P, 2], DT.bfloat16, name="negsm")

                # pass 1: scores + per-window padded segment max
                ppad = None
                for t in range(NT):
                    wi = t // NTH
                    tsl = slice(t * P, (t + 1) * P)
                    sg = w_sb["sres_g"][:, tsl]
                    srs = w_sb["sres_s"][:, tsl]
                    pk = ps.tile([P, P], DT.float32, tag="psq2", bufs=2)
                    nc.tensor.matmul(pk[:], lhsT=hT[:, tsl], rhs=w_sb["wk"][:],
                                     start=True, stop=False)
                    nc.tensor.matmul(pk[:], lhsT=w_sb["ones1"][:], rhs=w_sb["bk_row"][:],
                                     start=False, stop=True)
                    pqa = ps.tile([P, P], DT.float32, tag="pw2", bufs=2)
                    nc.tensor.matmul(pqa[:, :P], lhsT=sg,
                                     rhs=q_hi[:, wi * P:(wi + 1) * P],
                                     start=True, stop=False)
                    nc.tensor.matmul(pqa[:, :P], lhsT=sg,
                                     rhs=q_lo[:, wi * P:(wi + 1) * P],
                                     start=False, stop=True)
                    qa = wp.tile([P, P], DT.float32, tag="qa", bufs=2)
                    nc.vector.tensor_copy(qa[:], pqa[:, :P])
                    prod = wp.tile([P, P], DT.float32, tag="prod", bufs=2)
                    nc.vector.scalar_tensor_tensor(
                        prod[:], in0=pk[:], scalar=1.0, in1=qa[:],
                        op0=ALU.mult, op1=ALU.mult, accum_out=raw_st[:, t:t + 1])
                    ss = wp.tile([P, 32], DT.bfloat16, tag="ss", bufs=2)
                    nc.vector.tensor_scalar(ss[:], in0=w_sb["slot32"][:, t * 32:(t + 1) * 32],
                                            scalar1=raw_st[:, t:t + 1],
                                            scalar2=None, op0=ALU.mult)
                    if t % NTH == 0:
                        ppad = ps.tile([P, 32], DT.float32, tag="psq1", bufs=2)
                    nc.tensor.matmul(ppad[:], lhsT=srs, rhs=ss[:],
                                     start=(t % NTH == 0), stop=(t % NTH == NTH - 1))
                    if t % NTH == NTH - 1:
                        padded = wp.tile([P, 32], DT.float32, tag="padded", bufs=2)
                        nc.vector.tensor_tensor(padded[:], in0=ppad[:],
                                                in1=w_sb["padmask2"][:, wi * 32:(wi + 1) * 32],
                                                op=ALU.add)
                        nc.vector.tensor_reduce(negsm[:, wi:wi + 1], padded[:],
                                                axis=mybir.AxisListType.X, op=ALU.max,
                                                negate=True)

                # pass 2: exp weights, weighted v, per-residue sums
                ppool = pden = None
                den_sb = pp.tile([1, NLOC], DT.float32, name="den_sb")
                poolT = pp.tile([P, 2 * P], DT.float32, name="poolT")
                for t in range(NT):
                    wi = t // NTH
                    tsl = slice(t * P, (t + 1) * P)
                    sg = w_sb["sres_g"][:, tsl]
                    srs = w_sb["sres_s"][:, tsl]
                    pns = ps.tile([P, 1], DT.float32, tag="psq2", bufs=2)
                    nc.tensor.matmul(pns[:], lhsT=sg, rhs=negsm[:, wi:wi + 1],
                                     start=True, stop=True)
                    nsa = wp.tile([P, 1], DT.float32, tag="nsa", bufs=2)
                    nc.vector.tensor_scalar(nsa[:], in0=pns[:], scalar1=SQ,
                                            scalar2=None, op0=ALU.mult)
                    ex = wp.tile([P, 1], DT.float32, tag="ex", bufs=2)
                    nc.scalar.activation(ex[:], raw_st[:, t:t + 1], AF.Exp,
                                         bias=nsa[:], scale=SQ)
                    pv = ps.tile([P, P], DT.float32, tag="pw2", bufs=2)
                    nc.tensor.matmul(pv[:, :P], lhsT=hT[:, tsl], rhs=w_sb["wv"][:],
                                     start=True, stop=False)
                    nc.tensor.matmul(pv[:, :P], lhsT=w_sb["ones1"][:],
                                     rhs=w_sb["bv_row"][:], start=False, stop=True)
                    exv = wp.tile([P, P], DT.bfloat16, tag="exv", bufs=2)
                    nc.vector.tensor_scalar(exv[:], in0=pv[:, :P], scalar1=ex[:],
                                            scalar2=None, op0=ALU.mult)
                    ex_bf = wp.tile([P, 1], DT.bfloat16, tag="ex_bf", bufs=2)
                    nc.vector.tensor_copy(ex_bf[:], ex[:])
                    if t % NTH == 0:
                        ppool = ps.tile([P, P], DT.float32, tag="psq1", bufs=2)
                        pden = ps.tile([1, P], DT.float32, tag="pw1", bufs=2)
                    last = (t % NTH == NTH - 1)
                    nc.tensor.matmul(ppool[:], lhsT=exv[:], rhs=srs,
                                     start=(t % NTH == 0), stop=last)
                    nc.tensor.matmul(pden[:], lhsT=ex_bf[:], rhs=srs,
                                     start=(t % NTH == 0), stop=last)
                    if last:
                        nc.vector.reciprocal(den_sb[:, wi * P:(wi + 1) * P], pden[:])
                        pbc = ps.tile([P, P], DT.float32, tag="pw1", bufs=2)
                        nc.tensor.matmul(pbc[:], lhsT=w_sb["ones1"][:],
                                         rhs=den_sb[:, wi * P:(wi + 1) * P],
                                         start=True, stop=True)
                        bc = wp.tile([P, P], DT.float32, tag="bc", bufs=2)
                        nc.vector.tensor_copy(bc[:], pbc[:])
                        nc.vector.tensor_tensor(poolT[:, wi * P:(wi + 1) * P],
                                                in0=ppool[:], in1=bc[:], op=ALU.mult)

                # heads
                for wi in range(2):
                    osl = slice(wi * P, (wi + 1) * P)
                    pmu = ps.tile([32, P], DT.float32, tag="psq2", bufs=2)
                    nc.tensor.matmul(pmu[:], lhsT=w_sb["wmu"][:], rhs=poolT[:, osl],
                                     start=True, stop=True)
                    nc.scalar.activation(out_st[0:32, osl], pmu[:],
                                         AF.Identity, bias=w_sb["bmu_col"][:])
                    plv = ps.tile([32, P], DT.float32, tag="pw2", bufs=2)
                    nc.tensor.matmul(plv[:, :P], lhsT=w_sb["wlv"][:], rhs=poolT[:, osl],
                                     start=True, stop=True)
                    lvt = wp.tile([32, P], DT.float32, tag="lvt", bufs=2)
                    nc.scalar.activation(lvt[:], plv[:, :P], AF.Identity,
                                         bias=w_sb["blv_col"][:])
                    nc.vector.tensor_scalar(out_st[32:64, osl],
                                            in0=lvt[:], scalar1=2.0, scalar2=-10.0,
                                            op0=ALU.min, op1=ALU.max)
            nc.sync.dma_start(out_ext[:], out_st[:])

    nc.compile()
    return nc


# --------------------------------------------------------------------------
# entry point
# --------------------------------------------------------------------------

def kernel(**inputs):
    meta, per_core, wshared = _prep(inputs)
    key = (meta["e_pad"], tuple(meta["tiles"].ravel()))
    if key not in _cache:
        shapes = dict(wshared)
        shapes.update({k: v for k, v in per_core[0].items()})
        _cache[key] = _build(meta, shapes)
    nc = _cache[key]
    in_maps = []
    for c in range(C):
        m = dict(wshared)
        m.update(per_core[c])
        in_maps.append(m)
    trace = bool(int(os.environ.get("KERNEL_TRACE", "0")))
    r = run_bass_kernel_spmd(nc, in_maps, core_ids=list(range(C)), trace=trace)
    kernel.last_exec_ns = getattr(r, "exec_time_ns", None)
    kernel.last_results = r
    mu = np.concatenate([r.results[c]["out"][0:32, :].T for c in range(C)], 0)
    lv = np.concatenate([r.results[c]["out"][32:64, :].T for c in range(C)], 0)
    return mu.astype(np.float32), lv.astype(np.float32)

